# revision 1
# baseline (speedup 1.0000x reference)
"""AgentAttention kernel for 8 axon-tunneled TRN2 NeuronCores.

Strategy (per spec sharding_hint): data-parallel over batch. The full batch
B=64 is split into 8 shards of 8; each NeuronCore runs the whole
AgentAttention forward on its shard with all params replicated. Outputs are
gathered back into the full (64, 785, 768) array.

Neuron-friendly graph choices:
  - positional-bias tensors (bilinear resize + concat) are precomputed on CPU
    once per call (tiny), so the device graph sees two dense bias tensors;
  - the 768-group depthwise 3x3 conv is expressed as 9 shifted elementwise
    multiply-adds (grouped conv lowers terribly on neuron);
  - the scatter-add of the conv result is expressed as slice + concat.

Self-contained: hardcodes all shapes; reads nothing from disk.
"""

import numpy as np
import jax
import jax.numpy as jnp

DIM = 768
NUM_HEADS = 12
AGENT_NUM = 49
WINDOW = 28
POOL = 7
B = 64
N = 1 + WINDOW * WINDOW  # 785
N_CORES = 8


def _forward(x, w_qkv, w_proj, b_proj, dwc_w, dwc_b, pos_bias, agent_bias):
    """AgentAttention forward on one batch shard (b, N, C)."""
    b, n, c = x.shape
    H, hd, A, hw = NUM_HEADS, DIM // NUM_HEADS, AGENT_NUM, WINDOW
    scale = hd ** -0.5

    qkv = x @ w_qkv.T                                # (b, n, 3c)
    q, k, v = jnp.split(qkv, 3, axis=-1)

    s = hw // POOL
    qi = q[:, 1:, :].reshape(b, POOL, s, POOL, s, c)
    agent = qi.mean(axis=(2, 4)).reshape(b, A, c)    # (b, A, c)

    def heads(t, L):
        return t.reshape(b, L, H, hd).transpose(0, 2, 1, 3)

    qh = heads(q, n)
    kh = heads(k, n)
    vh = heads(v, n)
    agenth = heads(agent, A)                         # (b, H, A, hd)

    # stage 1: agents attend to K/V   (pos_bias: (1, H, A, n), precomputed)
    agent_attn = jax.nn.softmax(
        jnp.einsum('bhad,bhnd->bhan', agenth * scale, kh) + pos_bias, axis=-1)
    agent_v = agent_attn @ vh                        # (b, H, A, hd)

    # stage 2: queries attend to agents  (agent_bias: (1, H, n, A))
    q_attn = jax.nn.softmax(
        jnp.einsum('bhnd,bhad->bhna', qh * scale, agenth) + agent_bias, axis=-1)
    out = (q_attn @ agent_v).transpose(0, 2, 1, 3).reshape(b, n, c)

    # depthwise 3x3 conv on V image tokens as 9 shifted multiply-adds
    vi = vh[:, :, 1:, :].transpose(0, 2, 1, 3).reshape(b, hw, hw, c)
    vp = jnp.pad(vi, ((0, 0), (1, 1), (1, 1), (0, 0)))
    dw = dwc_b.astype(jnp.float32) * jnp.ones((b, hw, hw, c), jnp.float32)
    for dh in range(3):
        for dwi in range(3):
            dw = dw + vp[:, dh:dh + hw, dwi:dwi + hw, :] * dwc_w[dh, dwi, 0, :]

    out_img = out[:, 1:, :] + dw.reshape(b, hw * hw, c)
    out = jnp.concatenate([out[:, :1, :], out_img], axis=1)

    return out @ w_proj.T + b_proj


_pmapped = None
_NUM_DEV_ARGS = 8  # x + 7 params


def _axon_devices():
    try:
        devs = jax.devices("axon")
    except Exception:
        devs = [d for d in jax.devices() if d.platform != "cpu"] or jax.devices()
    return devs[:N_CORES]


def _get_pmapped():
    global _pmapped
    if _pmapped is None:
        _pmapped = jax.pmap(
            _forward,
            in_axes=(0,) + (None,) * (_NUM_DEV_ARGS - 1),
            devices=_axon_devices(),
        )
    return _pmapped


def _precompute_biases(inp):
    """CPU: bilinear-resize + assemble the two dense bias tensors."""
    cpu = jax.devices("cpu")[0]
    H, A, hw = NUM_HEADS, AGENT_NUM, WINDOW
    with jax.default_device(cpu):
        an = jnp.asarray(np.asarray(inp["an_bias"]))
        na = jnp.asarray(np.asarray(inp["na_bias"]))
        pb1 = jax.image.resize(an, (H, A, hw, hw), method="bilinear")
        pb1 = pb1.reshape(1, H, A, hw * hw)
        pb2 = (np.asarray(inp["ah_bias"]) + np.asarray(inp["aw_bias"])).reshape(
            1, H, A, hw * hw)
        pos_bias = jnp.concatenate(
            [jnp.asarray(np.asarray(inp["ac_bias"])), pb1 + pb2], axis=-1)

        ab1 = jax.image.resize(na, (H, A, hw, hw), method="bilinear")
        ab1 = ab1.reshape(1, H, A, hw * hw).transpose(0, 1, 3, 2)
        ab2 = (np.asarray(inp["ha_bias"]) + np.asarray(inp["wa_bias"])).reshape(
            1, H, hw * hw, A)
        agent_bias = jnp.concatenate(
            [jnp.asarray(np.asarray(inp["ca_bias"])), ab1 + ab2], axis=-2)
        return np.asarray(pos_bias), np.asarray(agent_bias)


def kernel(**inputs) -> np.ndarray:
    x = np.asarray(inputs["x"], dtype=np.float32)
    pos_bias, agent_bias = _precompute_biases(inputs)
    params = (
        np.asarray(inputs["w_qkv"], np.float32),
        np.asarray(inputs["w_proj"], np.float32),
        np.asarray(inputs["b_proj"], np.float32),
        np.asarray(inputs["dwc_w"], np.float32),
        np.asarray(inputs["dwc_b"], np.float32),
        pos_bias,
        agent_bias,
    )
    shards = x.reshape(N_CORES, B // N_CORES, N, DIM)
    fn = _get_pmapped()
    out = fn(shards, *params)
    return np.asarray(out).reshape(B, N, DIM)


if __name__ == "__main__":
    rng = np.random.default_rng(0)
    fake = {
        "x": rng.standard_normal((B, N, DIM), dtype=np.float32),
        "w_qkv": rng.standard_normal((3 * DIM, DIM), dtype=np.float32) * DIM ** -0.5,
        "w_proj": rng.standard_normal((DIM, DIM), dtype=np.float32) * DIM ** -0.5,
        "b_proj": np.zeros((DIM,), dtype=np.float32),
        "dwc_w": rng.standard_normal((3, 3, 1, DIM), dtype=np.float32) * 0.1,
        "dwc_b": np.zeros((DIM,), dtype=np.float32),
        "an_bias": rng.standard_normal((NUM_HEADS, AGENT_NUM, 7, 7), dtype=np.float32) * 0.02,
        "ah_bias": rng.standard_normal((1, NUM_HEADS, AGENT_NUM, WINDOW, 1), dtype=np.float32) * 0.02,
        "aw_bias": rng.standard_normal((1, NUM_HEADS, AGENT_NUM, 1, WINDOW), dtype=np.float32) * 0.02,
        "na_bias": rng.standard_normal((NUM_HEADS, AGENT_NUM, 7, 7), dtype=np.float32) * 0.02,
        "ha_bias": rng.standard_normal((1, NUM_HEADS, WINDOW, 1, AGENT_NUM), dtype=np.float32) * 0.02,
        "wa_bias": rng.standard_normal((1, NUM_HEADS, 1, WINDOW, AGENT_NUM), dtype=np.float32) * 0.02,
        "ac_bias": rng.standard_normal((1, NUM_HEADS, AGENT_NUM, 1), dtype=np.float32) * 0.02,
        "ca_bias": rng.standard_normal((1, NUM_HEADS, 1, AGENT_NUM), dtype=np.float32) * 0.02,
    }
    out = kernel(**fake)
    print("kernel out", out.shape, out.dtype, float(np.abs(out).mean()))

